# revision 14
# baseline (speedup 1.0000x reference)
"""AnomalyDAE 4-layer GCN on 8 TRN2 NeuronCores.

Strategy (node sharding per the sharding hint):
  - Nodes partitioned contiguously across 8 cores (6250/core).
  - Per layer: local matmul h = A @ W (PE, bf16 in / f32 acc), pre-scaled by
    dinv so symmetric normalization becomes out = dinv * segsum(hs[src]),
    hs = dinv * h.  AllGather the bf16 hs table -> [N, dout] per core.
  - Message passing: edges are deduplicated per (dst tile, src) on the host
    and sorted by destination tile; self-loops are eliminated (folded into
    the epilogue).  Per tile: dma_gather the unique src rows (bf16) from the
    table, then segment-sum ON THE TENSOR ENGINE: a multi-hot matrix
    Seg[tok, dst] (bf16, counts duplicate edges) and matmul(Seg^T @ msg)
    accumulates the tile's [128, dout] result in PSUM.
  - Seg is GENERATED ON-CHIP (DVE is_equal of an iota row vs per-token dst
    offsets) instead of streamed from DRAM: tokens carry up to two dst
    offsets (dstoff1/dstoff2, 255 = none); tokens with more dsts are split
    into copies host-side.  Multi-dst tokens are packed first in each
    segment so pass 2 touches only the leading blocks.
  - Each (tile, half) gather is SPLIT IN TWO sub-calls so one call's
    descriptors fit the 2048-desc SWDGE ring with headroom (a full 2176-tok
    call did not), letting all 4 SWDGE queues generate descriptors
    concurrently instead of drain-serializing.
  - Epilogue straight from PSUM: y = dinv*(psum + hs_tile) + b (the hs term
    is the self-loop), relu, PE-transpose to build the next layer's lhsT.
  - int16 gather indices => the feature table is split in two halves (per
    core: rows [0,3200) -> table A, rest -> table B), each AllGathered by a
    SEPARATE collective so half-A gathers overlap the half-B collective.
    The d=64 layer is zero-padded to 128 features (bf16 elem size must be
    a multiple of 256 bytes).
"""

import math

import numpy as np

N_CORES = 8
P = 128
import os as _os
SPLIT_CALLS = not _os.environ.get("GCN_NOSPLIT")

_CACHE = {}
_PRE_CACHE = {}
LAST_EXEC_NS = None


def _build_gcn(n_nodes, npc, npad, dims, layout, total_tok, n_cores):
    import concourse.bacc as bacc
    import concourse.tile as tile
    from concourse import mybir
    from concourse.library_config import mlp
    from contextlib import ExitStack

    f32 = mybir.dt.float32
    bf16 = mybir.dt.bfloat16
    i16 = mybir.dt.int16
    ntiles = npad // P
    nJ = total_tok // P

    nc = bacc.Bacc(
        "TRN2",
        debug=False,
        num_devices=n_cores,
        num_swdge_queues=4,
        dynamic_dma_scratch_size=32768,
    )

    din0 = dims[0][0]
    kc0 = math.ceil(din0 / P)
    xT = nc.declare_dram_parameter("xT", [din0, npad], bf16, isOutput=False)
    Ws = [
        nc.declare_dram_parameter(f"w{i}", [din, dout], bf16, isOutput=False)
        for i, (din, dout) in enumerate(dims)
    ]
    Bs = [
        nc.declare_dram_parameter(f"b{i}", [P, dout], bf16, isOutput=False)
        for i, (din, dout) in enumerate(dims)
    ]
    bsel_in = nc.declare_dram_parameter("bsel", [P, P], bf16, isOutput=False)
    dinv_in = nc.declare_dram_parameter("dinv", [P, ntiles], f32, isOutput=False)
    ident_in = nc.declare_dram_parameter("ident", [P, P], bf16, isOutput=False)
    gi_in = nc.declare_dram_parameter(
        "gidx", [P, total_tok // 16], i16, isOutput=False
    )
    iota_in = nc.declare_dram_parameter("iota", [P, P], bf16, isOutput=False)
    do1_in = nc.declare_dram_parameter("do1", [P, nJ], f32, isOutput=False)
    do2_in = nc.declare_dram_parameter("do2", [P, nJ], f32, isOutput=False)
    dout_last = dims[-1][1]
    out_ext = nc.declare_dram_parameter("out", [npc, dout_last], f32, isOutput=True)

    split_t = (ntiles + 1) // 2
    rows_a = split_t * P            # per-core rows in half A (tile-aligned)
    rows_b = npc - rows_a           # per-core rows in half B
    hs_loc = [
        (
            nc.dram_tensor(f"hs_locA{i}", [rows_a, d], bf16),
            nc.dram_tensor(f"hs_locB{i}", [rows_b, d], bf16),
        )
        for i, (_, d) in enumerate(dims)
    ]
    hs_full = [
        (
            nc.dram_tensor(
                f"hs_fullA{i}", [n_cores * rows_a, d], bf16, addr_space="Shared"
            ),
            nc.dram_tensor(
                f"hs_fullB{i}", [n_cores * rows_b, d], bf16, addr_space="Shared"
            ),
        )
        for i, (_, d) in enumerate(dims)
    ]

    relu = mybir.ActivationFunctionType.Relu
    copyf = mybir.ActivationFunctionType.Copy
    iseq = mybir.AluOpType.is_equal

    with tile.TileContext(nc) as tc, ExitStack() as ctx:
        const = ctx.enter_context(tc.tile_pool(name="const", bufs=1))
        at_pool = ctx.enter_context(tc.tile_pool(name="acts", bufs=2))
        work = ctx.enter_context(tc.tile_pool(name="work", bufs=6))
        msgp = ctx.enter_context(tc.tile_pool(name="msg", bufs=12))
        segp = ctx.enter_context(tc.tile_pool(name="seg", bufs=5))
        sg2p = ctx.enter_context(tc.tile_pool(name="sg2", bufs=4))
        hslp = ctx.enter_context(tc.tile_pool(name="hsl", bufs=6))
        psum = ctx.enter_context(tc.tile_pool(name="psum", bufs=2, space="PSUM"))
        psacc = ctx.enter_context(tc.tile_pool(name="psacc", bufs=4, space="PSUM"))

        nc.gpsimd.load_library(mlp)

        # num_idxs register hoisting: one dedicated GPR per distinct token
        # count, written ONCE.  Passing a raw int makes every gather share a
        # single scratch reg, and the WAR on that reg serializes desc-gen
        # across SWDGE queues (measured ~12us MOVE stalls between gathers).
        ntok_vals = sorted({c1 - c0 for grp in layout for _, c0, c1 in grp["calls"]})
        ntok_regs = {v: nc.gpsimd.to_reg(v) for v in ntok_vals}

        ident = const.tile([P, P], bf16)
        nc.sync.dma_start(out=ident[:], in_=ident_in[:, :])
        # row-0 selector: bsel^T @ b_tile adds bias row b to every PSUM row
        bsel = const.tile([P, P], bf16)
        nc.sync.dma_start(out=bsel[:], in_=bsel_in[:, :])
        dinv_sb = const.tile([P, ntiles], f32)
        nc.sync.dma_start(out=dinv_sb[:], in_=dinv_in[:])
        # gather indices are identical across layers: resident in SBUF
        gidx_sb = const.tile([P, total_tok // 16], i16)
        nc.sync.dma_start(out=gidx_sb[:], in_=gi_in[:, :])
        # seg-generation constants: iota row + per-token dst offsets
        iota_sb = const.tile([P, P], bf16)
        nc.sync.dma_start(out=iota_sb[:], in_=iota_in[:, :])
        do1_sb = const.tile([P, nJ], f32)
        nc.sync.dma_start(out=do1_sb[:], in_=do1_in[:, :])
        do2_sb = const.tile([P, nJ], f32)
        nc.sync.dma_start(out=do2_sb[:], in_=do2_in[:, :])

        w_sb, b_sb = [], []
        for i, (din, dout) in enumerate(dims):
            kcs = math.ceil(din / P)
            wi = []
            for kc in range(kcs):
                rows = min(P, din - kc * P)
                wt = const.tile([rows, dout], bf16, tag=f"w{i}_{kc}")
                nc.sync.dma_start(out=wt[:], in_=Ws[i][kc * P : kc * P + rows, :])
                wi.append(wt)
            w_sb.append(wi)
            bt = const.tile([P, dout], bf16, tag=f"b{i}")
            nc.sync.dma_start(out=bt[:], in_=Bs[i][:, :])
            b_sb.append(bt)

        aT = []
        for kc in range(kc0):
            rows = min(P, din0 - kc * P)
            t_ = at_pool.tile([rows, npad], bf16, tag=f"aT_{kc}", name=f"aT0_{kc}")
            nc.sync.dma_start(out=t_[:], in_=xT[kc * P : kc * P + rows, :])
            aT.append(t_)

        def mm_tile(li, aT_li, t):
            # local matmul + dinv pre-scale -> bf16 hs table shard (tile t);
            # fires the half's AllGather as soon as its last tile is stored
            # so collectives overlap the PREVIOUS layer's gather phase.
            din, dout = dims[li]
            ps = psum.tile([P, dout], f32, tag="mm", name="mm")
            for kc in range(len(aT_li)):
                nc.tensor.matmul(
                    ps[:],
                    aT_li[kc][:, t * P : (t + 1) * P],
                    w_sb[li][kc][:],
                    start=(kc == 0),
                    stop=(kc == len(aT_li) - 1),
                )
            hs_t = work.tile([P, dout], bf16, tag="hs", name="hs")
            nc.scalar.activation(
                hs_t[:], ps[:], copyf, bias=0.0, scale=dinv_sb[:, t : t + 1]
            )
            if t < split_t:
                r0 = t * P
                r1 = min((t + 1) * P, rows_a)
                dst = hs_loc[li][0]
            else:
                r0 = t * P - rows_a
                r1 = min((t + 1) * P - rows_a, rows_b)
                dst = hs_loc[li][1]
            nc.sync.dma_start(out=dst[r0:r1, :], in_=hs_t[0 : r1 - r0, :])
            for hh, t_last in ((0, split_t - 1), (1, ntiles - 1)):
                if t == t_last:
                    nc.gpsimd.collective_compute(
                        "AllGather",
                        mybir.AluOpType.bypass,
                        replica_groups=[list(range(n_cores))],
                        ins=[hs_loc[li][hh][:, :]],
                        outs=[hs_full[li][hh][:, :]],
                    )

        # layer-0 table build (nothing to overlap with)
        for t in range(ntiles):
            mm_tile(0, aT, t)

        gather_count = [0]

        n_layers = len(dims)
        for li, (din, dout) in enumerate(dims):
            last = li == n_layers - 1

            # ---- gather + PE segment-sum per dst tile ----
            if not last:
                kcs_next = math.ceil(dout / P)
                aT_next = []
                for kc in range(kcs_next):
                    rows = min(P, dout - kc * P)
                    aT_next.append(
                        at_pool.tile(
                            [rows, npad],
                            bf16,
                            tag=f"aT_{kc}",
                            name=f"aT{li + 1}_{kc}",
                        )
                    )

            for grp in layout:
                # one dma_gather per sub-call; msgs[h] = list of
                # (msg_tile, first_block, nblocks) in block order.
                # Queue = global call count % 4 with msg bufs % 4 == 0 keeps
                # each msg pool slot pinned to one SWDGE queue (slot k always
                # gathers on queue k % 4) -- required for DMA sem/queue
                # affinity.
                msgs = {}
                for h, c0, c1 in grp["calls"]:
                    qn = gather_count[0] % 4
                    gather_count[0] += 1
                    ntok = c1 - c0
                    msg = msgp.tile([P, ntok // P, dout], bf16, tag="msg", name="msg")
                    nc.gpsimd.dma_gather(
                        msg[:],
                        hs_full[li][h][:, :],
                        gidx_sb[:, c0 // 16 : c1 // 16],
                        ntok,
                        ntok_regs[ntok],
                        dout,
                        single_packet=False,
                        queue_num=qn,
                    )
                    msgs.setdefault(h, []).append((msg, c0 // P, ntok // P))

                for t, branges in grp["tiles"]:
                    pacc = psacc.tile([P, dout], f32, tag="segacc", name="segacc")
                    # self-loop term as the chain's first matmul:
                    # pacc = I^T @ hs_t  (= hs_t), then edge matmuls add in
                    if t < split_t:
                        r0 = t * P
                        r1 = min((t + 1) * P, rows_a)
                        src_t = hs_loc[li][0]
                    else:
                        r0 = t * P - rows_a
                        r1 = min((t + 1) * P - rows_a, rows_b)
                        src_t = hs_loc[li][1]
                    hsl = hslp.tile([P, dout], bf16, tag="hsl", name="hsl")
                    if r1 - r0 < P:
                        # short last tile: zero the tail so the I^T @ hsl
                        # matmul never reads uninitialized SBUF (0*NaN=NaN
                        # would poison the whole PSUM tile)
                        nc.vector.memset(hsl[:], 0)
                    nc.scalar.dma_start(
                        out=hsl[0 : r1 - r0, :], in_=src_t[r0:r1, :]
                    )
                    # ---- on-chip Seg generation (DVE):
                    # seg[tok, d] = (dstoff1[tok] == d) [+ (dstoff2[tok] == d)]
                    # Per-block tensor_scalar (per-partition scalar operand)
                    # instead of a broadcast tensor_tensor: ONE read port, so
                    # the DVE does not steal the Q7-shared SBUF port while
                    # SWDGE desc-gen is running, and 16-bit tensor_scalar has
                    # the 4x perf-mode uops.
                    segs = []
                    for h, j0, j1, nb2 in branges:
                        nblk = j1 - j0
                        segm = segp.tile(
                            [P, nblk, P], bf16, tag="seg", name="seg"
                        )
                        for jj in range(nblk):
                            nc.vector.tensor_scalar(
                                segm[:, jj, :],
                                iota_sb[:, :],
                                do1_sb[:, j0 + jj : j0 + jj + 1],
                                None,
                                iseq,
                            )
                        if nb2:
                            tmp2 = sg2p.tile([P, nb2, P], bf16, tag="sg2", name="sg2")
                            for jj in range(nb2):
                                nc.vector.tensor_scalar(
                                    tmp2[:, jj, :],
                                    iota_sb[:, :],
                                    do2_sb[:, j0 + jj : j0 + jj + 1],
                                    None,
                                    iseq,
                                )
                            nc.vector.tensor_add(
                                segm[:, 0:nb2, :], segm[:, 0:nb2, :], tmp2[:]
                            )
                        segs.append((h, j0, j1, segm))
                    nb_tot = sum(j1 - j0 for _, j0, j1, _ in segs)
                    nc.tensor.matmul(
                        pacc[:], ident[:], hsl[:], start=True, stop=False
                    )
                    nc.tensor.matmul(
                        pacc[:], bsel[:], b_sb[li][:], start=False,
                        stop=(nb_tot == 0),
                    )
                    bi = 0
                    for h, j0, j1, segm in segs:
                        subs = msgs[h]
                        for j in range(j0, j1):
                            # locate the sub-call tile containing block j
                            for msg, base, nb in subs:
                                if base <= j < base + nb:
                                    break
                            nc.tensor.matmul(
                                pacc[:],
                                segm[:, j - j0, :],
                                msg[:, j - base, :],
                                start=False,
                                stop=(bi == nb_tot - 1),
                            )
                            bi += 1

                    # ---- epilogue: one fused ACT op straight from PSUM
                    # (bias already added in-PSUM via the bsel matmul) ----
                    if last:
                        y = work.tile([P, dout], f32, tag="y", name="y")
                        nc.scalar.activation(
                            y[:], pacc[:], copyf, bias=0.0,
                            scale=dinv_sb[:, t : t + 1],
                        )
                        ro0 = t * P
                        ro1 = min((t + 1) * P, npc)
                        nc.sync.dma_start(
                            out=out_ext[ro0:ro1, :], in_=y[0 : ro1 - ro0, :]
                        )
                    else:
                        a_t = work.tile([P, dout], bf16, tag="a", name="a")
                        nc.scalar.activation(
                            a_t[:], pacc[:], relu, bias=0.0,
                            scale=dinv_sb[:, t : t + 1],
                        )
                        for kc in range(kcs_next):
                            wcols = min(P, dout - kc * P)
                            pt = psum.tile([wcols, P], bf16, tag="tr", name="tr")
                            nc.tensor.transpose(
                                pt[:], a_t[:, kc * P : kc * P + wcols], ident[:]
                            )
                            nc.scalar.copy(
                                aT_next[kc][:, t * P : (t + 1) * P], pt[:]
                            )
                        # next layer's table build for this tile, so the
                        # AllGather fires mid-way through THIS layer's
                        # gather phase instead of serializing after it
                        mm_tile(li + 1, aT_next, t)
            if not last:
                aT = aT_next

    nc.compile()
    return nc


def _preprocess(x, edge_index, n_nodes, npc, npad, n_cores, dims):
    import ml_dtypes

    src = np.asarray(edge_index[0], dtype=np.int64)
    dst = np.asarray(edge_index[1], dtype=np.int64)
    deg = np.bincount(dst, minlength=n_nodes).astype(np.float32) + 1.0
    dinv = (1.0 / np.sqrt(deg)).astype(np.float32)

    ntiles = npad // P
    split_t = (ntiles + 1) // 2
    rows_a = split_t * P
    rows_b = npc - rows_a
    nseg = ntiles * 2

    # Per core: dedup edges per (dst tile, src); token = unique (tile, src)
    # with up to TWO dst slots (dstoff1/dstoff2); tokens with more dst
    # entries are split into copies.  Multi-dst tokens go first in each
    # segment so the DVE pass-2 only touches the leading blocks.
    per_core = []
    for i in range(n_cores):
        lo = i * npc
        sel = (dst >= lo) & (dst < lo + npc)
        s = src[sel]
        dr = dst[sel] - lo
        tl = dr // P
        droff = dr - tl * P
        owner = s // npc
        rloc = s - owner * npc
        hh = (rloc >= rows_a).astype(np.int64)
        sg = np.where(hh == 0, owner * rows_a + rloc, owner * rows_b + rloc - rows_a)
        segid = tl * 2 + hh
        order = np.lexsort((droff, sg, segid))
        segid, sg, droff = segid[order], sg[order], droff[order]
        # unique (segid, sg) groups -> emit one token per 2 dst entries
        new_grp = np.ones(len(sg), bool)
        if len(sg) > 1:
            new_grp[1:] = (sg[1:] != sg[:-1]) | (segid[1:] != segid[:-1])
        grp_id = np.cumsum(new_grp) - 1
        grp_start = np.flatnonzero(new_grp)
        grp_len = np.diff(np.append(grp_start, len(sg)))
        # token copy index within group: edge e is entry (e - grp_start[g])
        ent = np.arange(len(sg)) - grp_start[grp_id]
        cp = ent // 2          # which token copy
        slot_in_tok = ent % 2  # dstoff1 or dstoff2
        # tokens: (segid, sg, copy) with n_entries = 1 or 2
        tok_first_edge = np.flatnonzero((slot_in_tok == 0))
        tok_seg = segid[tok_first_edge]
        tok_src = sg[tok_first_edge]
        tok_multi = np.zeros(len(tok_first_edge), bool)
        # token has 2 entries if the next edge belongs to the same token
        nf = tok_first_edge + 1
        ok = nf < len(sg)
        tok_multi[ok] = (grp_id[nf[ok]] == grp_id[tok_first_edge[ok]]) & (
            cp[nf[ok]] == cp[tok_first_edge[ok]]
        )
        d1 = droff[tok_first_edge]
        d2 = np.full(len(tok_first_edge), 255, np.int64)
        d2[tok_multi] = droff[tok_first_edge[tok_multi] + 1]
        per_core.append((tok_seg, tok_src, d1, d2, tok_multi))

    # per-segment layout: multi tokens first, then singles; pad to 128
    seglen = np.zeros(nseg, np.int64)
    nmulti = np.zeros(nseg, np.int64)
    for tok_seg, tok_src, d1, d2, tok_multi in per_core:
        cnt = np.bincount(tok_seg, minlength=nseg)
        m = np.bincount(tok_seg[tok_multi], minlength=nseg)
        seglen = np.maximum(seglen, cnt)
        nmulti = np.maximum(nmulti, m)
    seglen = ((seglen + P - 1) // P) * P
    nb2_seg = (nmulti + P - 1) // P

    # stream layout: one dst tile per group; each (tile, half) segment is
    # split into two sub-calls (ring-capacity + queue-concurrency)
    layout = []
    pos = 0
    seg_pos = np.zeros(nseg, np.int64)
    for t in range(ntiles):
        calls = []
        branges = []
        for h in (0, 1):
            sid = t * 2 + h
            L = int(seglen[sid])
            seg_pos[sid] = pos
            if L:
                branges.append((h, pos // P, (pos + L) // P, int(nb2_seg[sid])))
                nb = L // P
                nbA = (nb + 1) // 2 if SPLIT_CALLS else nb
                calls.append((h, pos, pos + nbA * P))
                if nb > nbA:
                    calls.append((h, pos + nbA * P, pos + L))
            pos += L
        layout.append({"calls": calls, "tiles": [(t, branges)]})
    total_tok = pos
    nJ = total_tok // P

    in_maps = []
    for i in range(n_cores):
        tok_seg, tok_src, d1, d2, tok_multi = per_core[i]
        # slot assignment: multi tokens first within each segment
        order = np.lexsort((~tok_multi, tok_seg))  # multi (True) first
        tok_seg, tok_src, d1, d2 = (
            tok_seg[order], tok_src[order], d1[order], d2[order]
        )
        ntok_seg = np.bincount(tok_seg, minlength=nseg)
        seg_first = np.zeros(nseg, np.int64)
        seg_first[1:] = np.cumsum(ntok_seg)[:-1]
        tok_slot = seg_pos[tok_seg] + (np.arange(len(tok_seg)) - seg_first[tok_seg])
        gidx = np.zeros(total_tok, np.int16)
        gidx[tok_slot] = tok_src.astype(np.int16)
        do1 = np.full(total_tok, 255, np.int64)
        do2 = np.full(total_tok, 255, np.int64)
        do1[tok_slot] = d1
        do2[tok_slot] = d2

        lo = i * npc
        x_loc = np.asarray(x[lo : lo + npc], dtype=np.float32)
        xT = np.zeros((x.shape[1], npad), dtype=ml_dtypes.bfloat16)
        xT[:, :npc] = x_loc.T.astype(ml_dtypes.bfloat16)
        dv = np.ones(npad, dtype=np.float32)
        dv[:npc] = dinv[lo : lo + npc]
        bsel = np.zeros((P, P), np.float32)
        bsel[0, :] = 1.0
        in_maps.append(
            {
                "xT": xT,
                "ident": np.eye(P, dtype=ml_dtypes.bfloat16),
                "bsel": bsel.astype(ml_dtypes.bfloat16),
                "dinv": np.ascontiguousarray(dv.reshape(ntiles, P).T),
                "gidx": np.tile(
                    np.ascontiguousarray(gidx.reshape(total_tok // 16, 16).T),
                    (8, 1),
                ),
                "iota": np.tile(
                    np.arange(P, dtype=np.float32), (P, 1)
                ).astype(ml_dtypes.bfloat16),
                "do1": np.ascontiguousarray(
                    do1.reshape(nJ, P).T
                ).astype(np.float32),
                "do2": np.ascontiguousarray(
                    do2.reshape(nJ, P).T
                ).astype(np.float32),
            }
        )
    return in_maps, layout, total_tok, dinv


def _pad_w(w, din_p, dout_p):
    out = np.zeros((din_p, dout_p), np.float32)
    out[: w.shape[0], : w.shape[1]] = w
    return out


def kernel(x, edge_index, W1, b1, W2, b2, W3, b3, W4, b4, **_unused):
    import ml_dtypes
    from concourse.bass_utils import run_bass_kernel_spmd

    x = np.asarray(x, dtype=np.float32)
    n_nodes = x.shape[0]
    npc = n_nodes // N_CORES
    ntiles = math.ceil(npc / P)
    npad = ntiles * P

    ws_raw = [np.asarray(w, np.float32) for w in (W1, W2, W3, W4)]
    bs_raw = [np.asarray(b, np.float32) for b in (b1, b2, b3, b4)]
    # pad every dim (except the first input / last output) to a multiple
    # of 128 so bf16 gather elem sizes stay multiples of 256B
    d_in = [ws_raw[0].shape[0]] + [
        max(P, math.ceil(w.shape[1] / P) * P) for w in ws_raw[:-1]
    ]
    d_last = max(P, math.ceil(ws_raw[-1].shape[1] / P) * P)
    d_out = d_in[1:] + [d_last]
    dims = list(zip(d_in, d_out))
    dout_raw = ws_raw[-1].shape[1]
    ws = [
        _pad_w(w, di, do).astype(ml_dtypes.bfloat16)
        for w, (di, do) in zip(ws_raw, dims)
    ]
    bs = [
        np.pad(b, (0, do - b.shape[0])).astype(np.float32)
        for b, (_, do) in zip(bs_raw, dims)
    ]

    pkey = (
        n_nodes,
        edge_index.shape[1],
        int(np.asarray(edge_index[:, :128]).sum()),
        float(x[:2].sum()),
    )
    if pkey not in _PRE_CACHE:
        _PRE_CACHE.clear()
        _PRE_CACHE[pkey] = _preprocess(
            x, edge_index, n_nodes, npc, npad, N_CORES, dims
        )
    in_maps, layout, total_tok, _ = _PRE_CACHE[pkey]
    key = (n_nodes, tuple(dims), total_tok)
    if key not in _CACHE:
        _CACHE[key] = _build_gcn(
            n_nodes, npc, npad, dims, layout, total_tok, N_CORES
        )
    nc = _CACHE[key]

    for m in in_maps:
        for i in range(4):
            m[f"w{i}"] = ws[i]
            brow = np.zeros((P, bs[i].shape[0]), np.float32)
            brow[0, :] = bs[i]
            m[f"b{i}"] = brow.astype(ml_dtypes.bfloat16)

    import os

    if os.environ.get("GCN_SIM"):
        from concourse.bass_interp import MultiCoreSim

        sim = MultiCoreSim(nc, N_CORES)
        for i in range(N_CORES):
            for k, v in in_maps[i].items():
                sim.cores[i].tensor(k)[:] = v
        sim.simulate(check_with_hw=False)
        return np.concatenate(
            [sim.cores[i].mem_tensor("out") for i in range(N_CORES)], axis=0
        )[:, :dout_raw]

    trace = bool(os.environ.get("GCN_TRACE"))
    res = run_bass_kernel_spmd(
        nc, in_maps, core_ids=list(range(N_CORES)), trace=trace
    )
    global LAST_EXEC_NS
    LAST_EXEC_NS = res.exec_time_ns
    return np.concatenate(
        [res.results[i]["out"] for i in range(N_CORES)], axis=0
    )[:, :dout_raw]


# revision 15
# speedup vs baseline: 1.2886x; 1.2886x over previous
"""AnomalyDAE 4-layer GCN on 8 TRN2 NeuronCores.

Strategy (node sharding per the sharding hint):
  - Nodes partitioned contiguously across 8 cores (6250/core).
  - Per layer: local matmul h = A @ W (PE, bf16 in / f32 acc), pre-scaled by
    dinv so symmetric normalization becomes out = dinv * segsum(hs[src]),
    hs = dinv * h.  AllGather the bf16 hs table -> [N, dout] per core.
  - Message passing: edges are deduplicated per (dst tile, src) on the host
    and sorted by destination tile; self-loops are eliminated (folded into
    the epilogue).  Per tile: dma_gather the unique src rows (bf16) from the
    table, then segment-sum ON THE TENSOR ENGINE: a multi-hot matrix
    Seg[tok, dst] (bf16, counts duplicate edges) and matmul(Seg^T @ msg)
    accumulates the tile's [128, dout] result in PSUM.
  - Seg is GENERATED ON-CHIP (DVE is_equal of an iota row vs per-token dst
    offsets) instead of streamed from DRAM: tokens carry up to two dst
    offsets (dstoff1/dstoff2, 255 = none); tokens with more dsts are split
    into copies host-side.  Multi-dst tokens are packed first in each
    segment so pass 2 touches only the leading blocks.
  - Each (tile, half) gather is SPLIT IN TWO sub-calls so one call's
    descriptors fit the 2048-desc SWDGE ring with headroom (a full 2176-tok
    call did not), letting all 4 SWDGE queues generate descriptors
    concurrently instead of drain-serializing.
  - Epilogue straight from PSUM: y = dinv*(psum + hs_tile) + b (the hs term
    is the self-loop), relu, PE-transpose to build the next layer's lhsT.
  - int16 gather indices => the feature table is split in two halves (per
    core: rows [0,3200) -> table A, rest -> table B), each AllGathered by a
    SEPARATE collective so half-A gathers overlap the half-B collective.
    The d=64 layer is zero-padded to 128 features (bf16 elem size must be
    a multiple of 256 bytes).
"""

import math

import numpy as np

N_CORES = 8
P = 128
import os as _os
SPLIT_CALLS = not _os.environ.get("GCN_NOSPLIT")

_CACHE = {}
_PRE_CACHE = {}
LAST_EXEC_NS = None


def _build_gcn(n_nodes, npc, npad, dims, layout, total_tok, n_cores):
    import concourse.bacc as bacc
    import concourse.tile as tile
    from concourse import mybir
    from concourse.library_config import mlp
    from contextlib import ExitStack

    f32 = mybir.dt.float32
    bf16 = mybir.dt.bfloat16
    i16 = mybir.dt.int16
    ntiles = npad // P
    nJ = total_tok // P

    nc = bacc.Bacc(
        "TRN2",
        debug=False,
        num_devices=n_cores,
        num_swdge_queues=4,
        dynamic_dma_scratch_size=32768,
    )

    din0 = dims[0][0]
    kc0 = math.ceil(din0 / P)
    xT = nc.declare_dram_parameter("xT", [din0, npad], bf16, isOutput=False)
    Ws = [
        nc.declare_dram_parameter(f"w{i}", [din, dout], bf16, isOutput=False)
        for i, (din, dout) in enumerate(dims)
    ]
    Bs = [
        nc.declare_dram_parameter(f"b{i}", [P, dout], bf16, isOutput=False)
        for i, (din, dout) in enumerate(dims)
    ]
    bsel_in = nc.declare_dram_parameter("bsel", [P, P], bf16, isOutput=False)
    dinv_in = nc.declare_dram_parameter("dinv", [P, ntiles], f32, isOutput=False)
    ident_in = nc.declare_dram_parameter("ident", [P, P], bf16, isOutput=False)
    gi_in = nc.declare_dram_parameter(
        "gidx", [P, total_tok // 16], i16, isOutput=False
    )
    iota_in = nc.declare_dram_parameter("iota", [P, 1, P], bf16, isOutput=False)
    do1_in = nc.declare_dram_parameter("do1", [P, nJ, 1], bf16, isOutput=False)
    do2_in = nc.declare_dram_parameter("do2", [P, nJ, 1], bf16, isOutput=False)
    dout_last = dims[-1][1]
    out_ext = nc.declare_dram_parameter("out", [npc, dout_last], f32, isOutput=True)

    split_t = (ntiles + 1) // 2
    rows_a = split_t * P            # per-core rows in half A (tile-aligned)
    rows_b = npc - rows_a           # per-core rows in half B
    hs_loc = [
        (
            nc.dram_tensor(f"hs_locA{i}", [rows_a, d], bf16),
            nc.dram_tensor(f"hs_locB{i}", [rows_b, d], bf16),
        )
        for i, (_, d) in enumerate(dims)
    ]
    hs_full = [
        (
            nc.dram_tensor(
                f"hs_fullA{i}", [n_cores * rows_a, d], bf16, addr_space="Shared"
            ),
            nc.dram_tensor(
                f"hs_fullB{i}", [n_cores * rows_b, d], bf16, addr_space="Shared"
            ),
        )
        for i, (_, d) in enumerate(dims)
    ]

    relu = mybir.ActivationFunctionType.Relu
    copyf = mybir.ActivationFunctionType.Copy
    iseq = mybir.AluOpType.is_equal

    with tile.TileContext(nc) as tc, ExitStack() as ctx:
        const = ctx.enter_context(tc.tile_pool(name="const", bufs=1))
        at_pool = ctx.enter_context(tc.tile_pool(name="acts", bufs=2))
        work = ctx.enter_context(tc.tile_pool(name="work", bufs=6))
        msgp = ctx.enter_context(tc.tile_pool(name="msg", bufs=12))
        segp = ctx.enter_context(tc.tile_pool(name="seg", bufs=5))
        sg2p = ctx.enter_context(tc.tile_pool(name="sg2", bufs=4))
        hslp = ctx.enter_context(tc.tile_pool(name="hsl", bufs=6))
        psum = ctx.enter_context(tc.tile_pool(name="psum", bufs=2, space="PSUM"))
        psacc = ctx.enter_context(tc.tile_pool(name="psacc", bufs=4, space="PSUM"))

        nc.gpsimd.load_library(mlp)

        # num_idxs register hoisting: one dedicated GPR per distinct token
        # count, written ONCE.  Passing a raw int makes every gather share a
        # single scratch reg, and the WAR on that reg serializes desc-gen
        # across SWDGE queues (measured ~12us MOVE stalls between gathers).
        ntok_vals = sorted({c1 - c0 for grp in layout for _, c0, c1 in grp["calls"]})
        ntok_regs = {v: nc.gpsimd.to_reg(v) for v in ntok_vals}

        ident = const.tile([P, P], bf16)
        nc.sync.dma_start(out=ident[:], in_=ident_in[:, :])
        # row-0 selector: bsel^T @ b_tile adds bias row b to every PSUM row
        bsel = const.tile([P, P], bf16)
        nc.sync.dma_start(out=bsel[:], in_=bsel_in[:, :])
        dinv_sb = const.tile([P, ntiles], f32)
        nc.sync.dma_start(out=dinv_sb[:], in_=dinv_in[:])
        # gather indices are identical across layers: resident in SBUF
        gidx_sb = const.tile([P, total_tok // 16], i16)
        nc.sync.dma_start(out=gidx_sb[:], in_=gi_in[:, :])
        # seg-generation constants: iota row + per-token dst offsets
        iota_sb = const.tile([P, 1, P], bf16)
        nc.sync.dma_start(out=iota_sb[:], in_=iota_in[:, :, :])
        do1_sb = const.tile([P, nJ, 1], bf16)
        nc.sync.dma_start(out=do1_sb[:], in_=do1_in[:, :, :])
        do2_sb = const.tile([P, nJ, 1], bf16)
        nc.sync.dma_start(out=do2_sb[:], in_=do2_in[:, :, :])

        w_sb, b_sb = [], []
        for i, (din, dout) in enumerate(dims):
            kcs = math.ceil(din / P)
            wi = []
            for kc in range(kcs):
                rows = min(P, din - kc * P)
                wt = const.tile([rows, dout], bf16, tag=f"w{i}_{kc}")
                nc.sync.dma_start(out=wt[:], in_=Ws[i][kc * P : kc * P + rows, :])
                wi.append(wt)
            w_sb.append(wi)
            bt = const.tile([P, dout], bf16, tag=f"b{i}")
            nc.sync.dma_start(out=bt[:], in_=Bs[i][:, :])
            b_sb.append(bt)

        aT = []
        for kc in range(kc0):
            rows = min(P, din0 - kc * P)
            t_ = at_pool.tile([rows, npad], bf16, tag=f"aT_{kc}", name=f"aT0_{kc}")
            nc.sync.dma_start(out=t_[:], in_=xT[kc * P : kc * P + rows, :])
            aT.append(t_)

        def mm_tile(li, aT_li, t):
            # local matmul + dinv pre-scale -> bf16 hs table shard (tile t);
            # fires the half's AllGather as soon as its last tile is stored
            # so collectives overlap the PREVIOUS layer's gather phase.
            din, dout = dims[li]
            ps = psum.tile([P, dout], f32, tag="mm", name="mm")
            for kc in range(len(aT_li)):
                nc.tensor.matmul(
                    ps[:],
                    aT_li[kc][:, t * P : (t + 1) * P],
                    w_sb[li][kc][:],
                    start=(kc == 0),
                    stop=(kc == len(aT_li) - 1),
                )
            hs_t = work.tile([P, dout], bf16, tag="hs", name="hs")
            nc.scalar.activation(
                hs_t[:], ps[:], copyf, bias=0.0, scale=dinv_sb[:, t : t + 1]
            )
            if t < split_t:
                r0 = t * P
                r1 = min((t + 1) * P, rows_a)
                dst = hs_loc[li][0]
            else:
                r0 = t * P - rows_a
                r1 = min((t + 1) * P - rows_a, rows_b)
                dst = hs_loc[li][1]
            nc.sync.dma_start(out=dst[r0:r1, :], in_=hs_t[0 : r1 - r0, :])
            for hh, t_last in ((0, split_t - 1), (1, ntiles - 1)):
                if t == t_last:
                    nc.gpsimd.collective_compute(
                        "AllGather",
                        mybir.AluOpType.bypass,
                        replica_groups=[list(range(n_cores))],
                        ins=[hs_loc[li][hh][:, :]],
                        outs=[hs_full[li][hh][:, :]],
                    )

        # layer-0 table build (nothing to overlap with)
        for t in range(ntiles):
            mm_tile(0, aT, t)

        gather_count = [0]

        n_layers = len(dims)
        for li, (din, dout) in enumerate(dims):
            last = li == n_layers - 1

            # ---- gather + PE segment-sum per dst tile ----
            if not last:
                kcs_next = math.ceil(dout / P)
                aT_next = []
                for kc in range(kcs_next):
                    rows = min(P, dout - kc * P)
                    aT_next.append(
                        at_pool.tile(
                            [rows, npad],
                            bf16,
                            tag=f"aT_{kc}",
                            name=f"aT{li + 1}_{kc}",
                        )
                    )

            for grp in layout:
                # one dma_gather per sub-call; msgs[h] = list of
                # (msg_tile, first_block, nblocks) in block order.
                # Queue = global call count % 4 with msg bufs % 4 == 0 keeps
                # each msg pool slot pinned to one SWDGE queue (slot k always
                # gathers on queue k % 4) -- required for DMA sem/queue
                # affinity.
                msgs = {}
                for h, c0, c1 in grp["calls"]:
                    qn = gather_count[0] % 4
                    gather_count[0] += 1
                    ntok = c1 - c0
                    msg = msgp.tile([P, ntok // P, dout], bf16, tag="msg", name="msg")
                    nc.gpsimd.dma_gather(
                        msg[:],
                        hs_full[li][h][:, :],
                        gidx_sb[:, c0 // 16 : c1 // 16],
                        ntok,
                        ntok_regs[ntok],
                        dout,
                        single_packet=False,
                        queue_num=qn,
                    )
                    msgs.setdefault(h, []).append((msg, c0 // P, ntok // P))

                for t, branges in grp["tiles"]:
                    pacc = psacc.tile([P, dout], f32, tag="segacc", name="segacc")
                    # self-loop term as the chain's first matmul:
                    # pacc = I^T @ hs_t  (= hs_t), then edge matmuls add in
                    if t < split_t:
                        r0 = t * P
                        r1 = min((t + 1) * P, rows_a)
                        src_t = hs_loc[li][0]
                    else:
                        r0 = t * P - rows_a
                        r1 = min((t + 1) * P - rows_a, rows_b)
                        src_t = hs_loc[li][1]
                    hsl = hslp.tile([P, dout], bf16, tag="hsl", name="hsl")
                    if r1 - r0 < P:
                        # short last tile: zero the tail so the I^T @ hsl
                        # matmul never reads uninitialized SBUF (0*NaN=NaN
                        # would poison the whole PSUM tile)
                        nc.vector.memset(hsl[:], 0)
                    nc.scalar.dma_start(
                        out=hsl[0 : r1 - r0, :], in_=src_t[r0:r1, :]
                    )
                    # ---- on-chip Seg generation (DVE):
                    # seg[tok, d] = (dstoff1[tok] == d) [+ (dstoff2[tok] == d)]
                    # Per-block tensor_scalar (per-partition scalar operand)
                    # instead of a broadcast tensor_tensor: ONE read port, so
                    # the DVE does not steal the Q7-shared SBUF port while
                    # SWDGE desc-gen is running, and 16-bit tensor_scalar has
                    # the 4x perf-mode uops.
                    segs = []
                    for h, j0, j1, nb2 in branges:
                        nblk = j1 - j0
                        segm = segp.tile(
                            [P, nblk, P], bf16, tag="seg", name="seg"
                        )
                        nc.vector.tensor_tensor(
                            segm[:],
                            iota_sb[:].to_broadcast([P, nblk, P]),
                            do1_sb[:, j0:j1, :].to_broadcast([P, nblk, P]),
                            iseq,
                        )
                        if nb2:
                            tmp2 = sg2p.tile([P, nb2, P], bf16, tag="sg2", name="sg2")
                            nc.vector.tensor_tensor(
                                tmp2[:],
                                iota_sb[:].to_broadcast([P, nb2, P]),
                                do2_sb[:, j0 : j0 + nb2, :].to_broadcast(
                                    [P, nb2, P]
                                ),
                                iseq,
                            )
                            nc.vector.tensor_add(
                                segm[:, 0:nb2, :], segm[:, 0:nb2, :], tmp2[:]
                            )
                        segs.append((h, j0, j1, segm))
                    nb_tot = sum(j1 - j0 for _, j0, j1, _ in segs)
                    nc.tensor.matmul(
                        pacc[:], ident[:], hsl[:], start=True, stop=False
                    )
                    nc.tensor.matmul(
                        pacc[:], bsel[:], b_sb[li][:], start=False,
                        stop=(nb_tot == 0),
                    )
                    bi = 0
                    for h, j0, j1, segm in segs:
                        subs = msgs[h]
                        for j in range(j0, j1):
                            # locate the sub-call tile containing block j
                            for msg, base, nb in subs:
                                if base <= j < base + nb:
                                    break
                            nc.tensor.matmul(
                                pacc[:],
                                segm[:, j - j0, :],
                                msg[:, j - base, :],
                                start=False,
                                stop=(bi == nb_tot - 1),
                            )
                            bi += 1

                    # ---- epilogue: one fused ACT op straight from PSUM
                    # (bias already added in-PSUM via the bsel matmul) ----
                    if last:
                        y = work.tile([P, dout], f32, tag="y", name="y")
                        nc.scalar.activation(
                            y[:], pacc[:], copyf, bias=0.0,
                            scale=dinv_sb[:, t : t + 1],
                        )
                        ro0 = t * P
                        ro1 = min((t + 1) * P, npc)
                        nc.sync.dma_start(
                            out=out_ext[ro0:ro1, :], in_=y[0 : ro1 - ro0, :]
                        )
                    else:
                        a_t = work.tile([P, dout], bf16, tag="a", name="a")
                        nc.scalar.activation(
                            a_t[:], pacc[:], relu, bias=0.0,
                            scale=dinv_sb[:, t : t + 1],
                        )
                        for kc in range(kcs_next):
                            wcols = min(P, dout - kc * P)
                            pt = psum.tile([wcols, P], bf16, tag="tr", name="tr")
                            nc.tensor.transpose(
                                pt[:], a_t[:, kc * P : kc * P + wcols], ident[:]
                            )
                            nc.scalar.copy(
                                aT_next[kc][:, t * P : (t + 1) * P], pt[:]
                            )
                        # next layer's table build for this tile, so the
                        # AllGather fires mid-way through THIS layer's
                        # gather phase instead of serializing after it
                        mm_tile(li + 1, aT_next, t)
            if not last:
                aT = aT_next

    nc.compile()
    return nc


def _preprocess(x, edge_index, n_nodes, npc, npad, n_cores, dims):
    import ml_dtypes

    src = np.asarray(edge_index[0], dtype=np.int64)
    dst = np.asarray(edge_index[1], dtype=np.int64)
    deg = np.bincount(dst, minlength=n_nodes).astype(np.float32) + 1.0
    dinv = (1.0 / np.sqrt(deg)).astype(np.float32)

    ntiles = npad // P
    split_t = (ntiles + 1) // 2
    rows_a = split_t * P
    rows_b = npc - rows_a
    nseg = ntiles * 2

    # Per core: dedup edges per (dst tile, src); token = unique (tile, src)
    # with up to TWO dst slots (dstoff1/dstoff2); tokens with more dst
    # entries are split into copies.  Multi-dst tokens go first in each
    # segment so the DVE pass-2 only touches the leading blocks.
    per_core = []
    for i in range(n_cores):
        lo = i * npc
        sel = (dst >= lo) & (dst < lo + npc)
        s = src[sel]
        dr = dst[sel] - lo
        tl = dr // P
        droff = dr - tl * P
        owner = s // npc
        rloc = s - owner * npc
        hh = (rloc >= rows_a).astype(np.int64)
        sg = np.where(hh == 0, owner * rows_a + rloc, owner * rows_b + rloc - rows_a)
        segid = tl * 2 + hh
        order = np.lexsort((droff, sg, segid))
        segid, sg, droff = segid[order], sg[order], droff[order]
        # unique (segid, sg) groups -> emit one token per 2 dst entries
        new_grp = np.ones(len(sg), bool)
        if len(sg) > 1:
            new_grp[1:] = (sg[1:] != sg[:-1]) | (segid[1:] != segid[:-1])
        grp_id = np.cumsum(new_grp) - 1
        grp_start = np.flatnonzero(new_grp)
        grp_len = np.diff(np.append(grp_start, len(sg)))
        # token copy index within group: edge e is entry (e - grp_start[g])
        ent = np.arange(len(sg)) - grp_start[grp_id]
        cp = ent // 2          # which token copy
        slot_in_tok = ent % 2  # dstoff1 or dstoff2
        # tokens: (segid, sg, copy) with n_entries = 1 or 2
        tok_first_edge = np.flatnonzero((slot_in_tok == 0))
        tok_seg = segid[tok_first_edge]
        tok_src = sg[tok_first_edge]
        tok_multi = np.zeros(len(tok_first_edge), bool)
        # token has 2 entries if the next edge belongs to the same token
        nf = tok_first_edge + 1
        ok = nf < len(sg)
        tok_multi[ok] = (grp_id[nf[ok]] == grp_id[tok_first_edge[ok]]) & (
            cp[nf[ok]] == cp[tok_first_edge[ok]]
        )
        d1 = droff[tok_first_edge]
        d2 = np.full(len(tok_first_edge), 255, np.int64)
        d2[tok_multi] = droff[tok_first_edge[tok_multi] + 1]
        per_core.append((tok_seg, tok_src, d1, d2, tok_multi))

    # per-segment layout: multi tokens first, then singles; pad to 128
    seglen = np.zeros(nseg, np.int64)
    nmulti = np.zeros(nseg, np.int64)
    for tok_seg, tok_src, d1, d2, tok_multi in per_core:
        cnt = np.bincount(tok_seg, minlength=nseg)
        m = np.bincount(tok_seg[tok_multi], minlength=nseg)
        seglen = np.maximum(seglen, cnt)
        nmulti = np.maximum(nmulti, m)
    seglen = ((seglen + P - 1) // P) * P
    nb2_seg = (nmulti + P - 1) // P

    # stream layout: one dst tile per group; each (tile, half) segment is
    # split into two sub-calls (ring-capacity + queue-concurrency)
    layout = []
    pos = 0
    seg_pos = np.zeros(nseg, np.int64)
    for t in range(ntiles):
        calls = []
        branges = []
        for h in (0, 1):
            sid = t * 2 + h
            L = int(seglen[sid])
            seg_pos[sid] = pos
            if L:
                branges.append((h, pos // P, (pos + L) // P, int(nb2_seg[sid])))
                nb = L // P
                nbA = (nb + 1) // 2 if SPLIT_CALLS else nb
                calls.append((h, pos, pos + nbA * P))
                if nb > nbA:
                    calls.append((h, pos + nbA * P, pos + L))
            pos += L
        layout.append({"calls": calls, "tiles": [(t, branges)]})
    total_tok = pos
    nJ = total_tok // P

    in_maps = []
    for i in range(n_cores):
        tok_seg, tok_src, d1, d2, tok_multi = per_core[i]
        # slot assignment: multi tokens first within each segment
        order = np.lexsort((~tok_multi, tok_seg))  # multi (True) first
        tok_seg, tok_src, d1, d2 = (
            tok_seg[order], tok_src[order], d1[order], d2[order]
        )
        ntok_seg = np.bincount(tok_seg, minlength=nseg)
        seg_first = np.zeros(nseg, np.int64)
        seg_first[1:] = np.cumsum(ntok_seg)[:-1]
        tok_slot = seg_pos[tok_seg] + (np.arange(len(tok_seg)) - seg_first[tok_seg])
        gidx = np.zeros(total_tok, np.int16)
        gidx[tok_slot] = tok_src.astype(np.int16)
        do1 = np.full(total_tok, 255, np.int64)
        do2 = np.full(total_tok, 255, np.int64)
        do1[tok_slot] = d1
        do2[tok_slot] = d2

        lo = i * npc
        x_loc = np.asarray(x[lo : lo + npc], dtype=np.float32)
        xT = np.zeros((x.shape[1], npad), dtype=ml_dtypes.bfloat16)
        xT[:, :npc] = x_loc.T.astype(ml_dtypes.bfloat16)
        dv = np.ones(npad, dtype=np.float32)
        dv[:npc] = dinv[lo : lo + npc]
        bsel = np.zeros((P, P), np.float32)
        bsel[0, :] = 1.0
        in_maps.append(
            {
                "xT": xT,
                "ident": np.eye(P, dtype=ml_dtypes.bfloat16),
                "bsel": bsel.astype(ml_dtypes.bfloat16),
                "dinv": np.ascontiguousarray(dv.reshape(ntiles, P).T),
                "gidx": np.tile(
                    np.ascontiguousarray(gidx.reshape(total_tok // 16, 16).T),
                    (8, 1),
                ),
                "iota": np.tile(
                    np.arange(P, dtype=np.float32), (P, 1, 1)
                ).astype(ml_dtypes.bfloat16),
                "do1": np.ascontiguousarray(
                    do1.reshape(nJ, P).T.reshape(P, nJ, 1)
                ).astype(ml_dtypes.bfloat16),
                "do2": np.ascontiguousarray(
                    do2.reshape(nJ, P).T.reshape(P, nJ, 1)
                ).astype(ml_dtypes.bfloat16),
            }
        )
    return in_maps, layout, total_tok, dinv


def _pad_w(w, din_p, dout_p):
    out = np.zeros((din_p, dout_p), np.float32)
    out[: w.shape[0], : w.shape[1]] = w
    return out


def kernel(x, edge_index, W1, b1, W2, b2, W3, b3, W4, b4, **_unused):
    import ml_dtypes
    from concourse.bass_utils import run_bass_kernel_spmd

    x = np.asarray(x, dtype=np.float32)
    n_nodes = x.shape[0]
    npc = n_nodes // N_CORES
    ntiles = math.ceil(npc / P)
    npad = ntiles * P

    ws_raw = [np.asarray(w, np.float32) for w in (W1, W2, W3, W4)]
    bs_raw = [np.asarray(b, np.float32) for b in (b1, b2, b3, b4)]
    # pad every dim (except the first input / last output) to a multiple
    # of 128 so bf16 gather elem sizes stay multiples of 256B
    d_in = [ws_raw[0].shape[0]] + [
        max(P, math.ceil(w.shape[1] / P) * P) for w in ws_raw[:-1]
    ]
    d_last = max(P, math.ceil(ws_raw[-1].shape[1] / P) * P)
    d_out = d_in[1:] + [d_last]
    dims = list(zip(d_in, d_out))
    dout_raw = ws_raw[-1].shape[1]
    ws = [
        _pad_w(w, di, do).astype(ml_dtypes.bfloat16)
        for w, (di, do) in zip(ws_raw, dims)
    ]
    bs = [
        np.pad(b, (0, do - b.shape[0])).astype(np.float32)
        for b, (_, do) in zip(bs_raw, dims)
    ]

    pkey = (
        n_nodes,
        edge_index.shape[1],
        int(np.asarray(edge_index[:, :128]).sum()),
        float(x[:2].sum()),
    )
    if pkey not in _PRE_CACHE:
        _PRE_CACHE.clear()
        _PRE_CACHE[pkey] = _preprocess(
            x, edge_index, n_nodes, npc, npad, N_CORES, dims
        )
    in_maps, layout, total_tok, _ = _PRE_CACHE[pkey]
    key = (n_nodes, tuple(dims), total_tok)
    if key not in _CACHE:
        _CACHE[key] = _build_gcn(
            n_nodes, npc, npad, dims, layout, total_tok, N_CORES
        )
    nc = _CACHE[key]

    for m in in_maps:
        for i in range(4):
            m[f"w{i}"] = ws[i]
            brow = np.zeros((P, bs[i].shape[0]), np.float32)
            brow[0, :] = bs[i]
            m[f"b{i}"] = brow.astype(ml_dtypes.bfloat16)

    import os

    if os.environ.get("GCN_SIM"):
        from concourse.bass_interp import MultiCoreSim

        sim = MultiCoreSim(nc, N_CORES)
        for i in range(N_CORES):
            for k, v in in_maps[i].items():
                sim.cores[i].tensor(k)[:] = v
        sim.simulate(check_with_hw=False)
        return np.concatenate(
            [sim.cores[i].mem_tensor("out") for i in range(N_CORES)], axis=0
        )[:, :dout_raw]

    trace = bool(os.environ.get("GCN_TRACE"))
    res = run_bass_kernel_spmd(
        nc, in_maps, core_ids=list(range(N_CORES)), trace=trace
    )
    global LAST_EXEC_NS
    LAST_EXEC_NS = res.exec_time_ns
    return np.concatenate(
        [res.results[i]["out"] for i in range(N_CORES)], axis=0
    )[:, :dout_raw]
